# revision 23
# baseline (speedup 1.0000x reference)
"""CrossMerge kernel for Trainium2 (8 NeuronCores, data-parallel over batch).

Computation (per batch b):
    inv[k]  = stable argsort of vec_indices[b, :, k]              (k = 0, 1)
    s_k[u,d] = ys[b,k,d,u] + ys[b,k+2,d,L-1-u]   (fwd + flipped half, pre-summed)
    out[b,d,l] = sum_k s_k[inv[k][l], d]

All-SBUF design (no DRAM scratch roundtrip, bit-exact f32):
  Phase A (per b,k): stream ys[b,k]/[b,k+2] in l-column chunks, one vector
           add per chunk (second operand read reversed), TensorE-transpose
           128-wide l tiles into PSUM [l,d] f32, then split every f32 into
           its two 16-bit halves while copying PSUM->SBUF into a token-major
           table: token u at partition u%128, rank u//128, payload
           [lo d0..127 | hi d0..127 | lo d128..191 pad | hi d128..191 pad]
           (512 int16 units = 1024B). Pure byte movement -> exact.
  Phase B (per b,k): SBUF-source dma_gather with transpose=True pulls the
           permuted tokens straight from the SBUF table; the DMA transpose
           unit (16-bit granularity) lands unit c*128+p on partition p,
           chunk c - so the lo/hi planes come out partition-aligned in
           [d, l] orientation. No second TensorE transpose pass needed.
  Phase C: re-interleave lo/hi planes into f32 bit patterns (strided i16
           copies), one f32 add sums the k pair, store [d, l] slices.

Host does only sharding + argsort-derived index prep; all tensor data
movement/compute is on device.
"""
import sys

sys.path.insert(0, "/opt/trn_rl_repo")

import numpy as np

import concourse.bacc as bacc
import concourse.bass as bass
import concourse.mybir as mybir
import concourse.tile as tile
from concourse.bass_utils import run_bass_kernel_spmd
from concourse.masks import make_identity

# Problem constants (hardcoded per contract).
B, K, D, H, W = 16, 4, 192, 56, 56
L = H * W          # 3136
K2 = K // 2        # 2
NCORES = 8
BL = B // NCORES   # 2 batches per core
C = 25             # l tiles of 128: 24 full + 1 of 64
LP = C * 128       # 3200 padded
U = 512            # int16 units per token payload (1024 bytes)
NW = LP // 16      # idx columns in wrapped int16 layout (200)
CHUNK = 512        # phase-A l-column streaming chunk (4 c-tiles)
F32 = mybir.dt.float32
I16 = mybir.dt.int16

# gather chunks: c-block ranges [0,7) [7,13) [13,19) [19,25)
CH_CLO = (0, 7, 13, 19)
CH_N = (896, 768, 768, 768)          # idx positions per chunk (%128 == 0)
CH_VALID = (896, 768, 768, 704)      # non-pad idx count per chunk
CH_L0 = (0, 896, 1664, 2432)         # output l offset per chunk


def crossmerge_body(tc, out_ap, ys_ap, idx_ap):
    """Tile kernel body.

    out_ap: [BL, 192, 3136] f32 (ExternalOutput)
    ys_ap:  [BL, 4, 192, 3136] f32 (ExternalInput)
    idx_ap: [BL, 2, 128, 200] i16, plain inv tokens, 16-wrapped, x8 groups
    """
    nc = tc.nc

    with (
        tc.tile_pool(name="const", bufs=1) as cpool,
        tc.tile_pool(name="idx", bufs=4) as ipool,
        tc.tile_pool(name="y", bufs=2) as ypool,
        tc.tile_pool(name="z", bufs=2) as zpool,
        tc.tile_pool(name="s", bufs=2) as spool,
        tc.tile_pool(name="tab", bufs=4) as tabpool,
        tc.tile_pool(name="g", bufs=6) as gpool,
        tc.tile_pool(name="x", bufs=4) as xpool,
        tc.tile_pool(name="xt", bufs=4) as xtpool,
        tc.tile_pool(name="ps", bufs=4, space="PSUM") as pspool,
    ):
        ident = cpool.tile([128, 128], F32)
        make_identity(nc, ident[:])

        # idx tiles first in the sync queue; gathers need them early
        idx_tiles = []
        for b in range(BL):
            i0 = ipool.tile([128, NW], I16, tag="idx")
            i1 = ipool.tile([128, NW], I16, tag="idx")
            nc.sync.dma_start(out=i0[:], in_=idx_ap[b, 0])
            nc.sync.dma_start(out=i1[:], in_=idx_ap[b, 1])
            idx_tiles.append((i0, i1))

        tabs = {}

        def phase_a(b, k):
            """Build the SBUF token table for (b, k)."""
            tab = tabpool.tile([128, C * U], I16, tag="tab")
            tabs[(b, k)] = tab
            tabv = tab[:].rearrange("p (c u) -> p c u", u=U)
            # the gather reads whole 1024B payloads; zero the pad units and
            # the never-written rank-24 rows so no uninit bytes are read
            nc.gpsimd.memset(tabv[:, :, 320:384], 0.0)
            nc.gpsimd.memset(tabv[:, :, 448:512], 0.0)
            nc.gpsimd.memset(tabv[64:, C - 1, :], 0.0)
            ys_f = ys_ap[b, k].rearrange("(g p) l -> p g l", p=96)
            ys_r = ys_ap[b, k + K2].rearrange("(g p) l -> p g l", p=96)
            for lo in range(0, L, CHUNK):
                hi = min(lo + CHUNK, L)
                w = hi - lo
                yc = ypool.tile([96, 2 * w], F32, tag="y")
                zc = zpool.tile([96, 2 * w], F32, tag="z")
                ycv = yc[:].rearrange("p (g l) -> p g l", g=2)
                zcv = zc[:].rearrange("p (g l) -> p g l", g=2)
                nc.sync.dma_start(out=ycv, in_=ys_f[:, :, lo:hi])
                nc.sync.dma_start(out=zcv, in_=ys_r[:, :, L - hi:L - lo])
                # s[:, g, l] = y[:, g, l] + z[:, g, L-1-l]
                sc = spool.tile([96, 2 * w], F32, tag="s")
                scv = sc[:].rearrange("p (g l) -> p g l", g=2)
                nc.vector.tensor_add(out=scv, in0=ycv, in1=zcv[:, :, ::-1])
                c0 = lo // 128
                ncs = (w + 127) // 128
                for j0 in range(0, ncs, 2):
                    gn = min(2, ncs - j0)
                    cc = c0 + j0
                    # all c-tiles in a 2-group are full except c=24 (alone)
                    szg = min(128, L - cc * 128)
                    ps = pspool.tile([128, 2, 192], F32, space="PSUM")
                    for j in range(gn):
                        c = cc + j
                        off = c * 128 - lo
                        for g in range(2):
                            nc.tensor.transpose(
                                out=ps[:szg, j, 96 * g:96 * (g + 1)],
                                in_=scv[:, g, off:off + szg],
                                identity=ident[:96, :96],
                            )
                    # split f32 -> (lo16, hi16) while copying PSUM -> table;
                    # on the scalar engine: int16 roundtrips the activation
                    # f32 path exactly, and it keeps the vector engine free
                    psi = ps[:].bitcast(I16)  # [128, 2, 384]: x = 2d+parity
                    nc.scalar.copy(
                        out=tabv[:szg, cc:cc + gn, 0:128],
                        in_=psi[:szg, :gn, 0:256:2])
                    nc.scalar.copy(
                        out=tabv[:szg, cc:cc + gn, 128:256],
                        in_=psi[:szg, :gn, 1:256:2])
                    nc.scalar.copy(
                        out=tabv[:szg, cc:cc + gn, 256:320],
                        in_=psi[:szg, :gn, 256:384:2])
                    nc.scalar.copy(
                        out=tabv[:szg, cc:cc + gn, 384:448],
                        in_=psi[:szg, :gn, 257:384:2])

        gs = {}

        def phase_b(b, k):
            """Issue the 4 chunked SBUF-source gathers for (b, k).

            All transpose-gathers go on ONE SWDGE queue: concurrent
            multi-queue transpose drains corrupt the shared transpose RX
            unit on this runtime (verified empirically).
            """
            it = idx_tiles[b][k]
            for ch in range(4):
                n = CH_N[ch]
                col0 = CH_L0[ch] // 16
                gt = gpool.tile([128, 4 * n], I16, tag="g")
                gv = gt[:].rearrange("p (c i) -> p c i", c=4)
                nc.gpsimd.dma_gather(
                    out_ap=gv,
                    in_ap=tabs[(b, k)][:],
                    idxs_ap=it[:, col0:col0 + n // 16],
                    num_idxs=n, num_idxs_reg=CH_VALID[ch], elem_size=U,
                    transpose=True, single_packet=True, queue_num=0,
                    sbuf_tokens_per_rank=128,
                    sbuf_free_dim_per_rank=2 * U)
                gs[(b, k, ch)] = gt

        def phase_c(b):
            """Recombine lo/hi planes, sum the k pair, store."""
            ov = out_ap[b]
            for ch in range(4):
                n, valid, l0 = CH_N[ch], CH_VALID[ch], CH_L0[ch]
                parts = []
                for k in range(K2):
                    gv = gs[(b, k, ch)][:].rearrange("p (c i) -> p c i", c=4)
                    xm = xpool.tile([128, n], F32, tag="x")
                    xt = xtpool.tile([64, n], F32, tag="xt")
                    xmv = xm[:].bitcast(I16).rearrange(
                        "p (i t) -> p i t", t=2)
                    xtv = xt[:].bitcast(I16).rearrange(
                        "p (i t) -> p i t", t=2)
                    nc.vector.tensor_copy(out=xmv[:, :, 0], in_=gv[:, 0, :])
                    nc.vector.tensor_copy(out=xmv[:, :, 1], in_=gv[:, 1, :])
                    nc.vector.tensor_copy(out=xtv[:, :, 0], in_=gv[:64, 2, :])
                    nc.vector.tensor_copy(out=xtv[:, :, 1], in_=gv[:64, 3, :])
                    parts.append((xm, xt))
                (x0, t0), (x1, t1) = parts
                nc.vector.tensor_add(out=x0[:], in0=x0[:], in1=x1[:])
                nc.vector.tensor_add(out=t0[:], in0=t0[:], in1=t1[:])
                nc.scalar.dma_start(
                    out=ov[0:128, l0:l0 + valid], in_=x0[:, :valid])
                nc.scalar.dma_start(
                    out=ov[128:192, l0:l0 + valid], in_=t0[:, :valid])

        # schedule: descgen for (b,k) directly after its table so the
        # gpsimd descgen stream overlaps the next table's build
        phase_a(0, 0)
        phase_b(0, 0)
        phase_a(0, 1)
        phase_b(0, 1)
        phase_a(1, 0)
        phase_b(1, 0)
        # issue A/B(1,1) before C(0): C(0)'s drain-stalled vector ops would
        # otherwise delay table (1,1)'s adds and starve the descriptor ring
        phase_a(1, 1)
        phase_b(1, 1)
        phase_c(0)
        phase_c(1)


def _host_prep(ys, vec_indices):
    """Shard inputs and build gather index tensors."""
    ys = np.ascontiguousarray(np.asarray(ys, dtype=np.float32)).reshape(
        B, K, D, L)
    vi = np.asarray(vec_indices)
    inv = np.argsort(vi, axis=1, kind="stable")          # [B, L, K2]
    invt = np.transpose(inv, (0, 2, 1)).astype(np.int16)  # [B, K2, L]
    # pad to 3200 with -1 (trailing pad: gathered garbage is never stored),
    # wrap in 16 partitions, replicate to the 8 gpsimd core groups
    rpad = np.concatenate(
        [invt, np.full((B, K2, LP - L), -1, dtype=np.int16)], axis=2)
    w = rpad.reshape(B, K2, NW, 16).transpose(0, 1, 3, 2)  # [B, K2, 16, NW]
    w = np.tile(w, (1, 1, 8, 1))                           # [B, K2, 128, NW]
    in_maps = []
    for i in range(NCORES):
        in_maps.append({
            "ys": ys[BL * i:BL * (i + 1)],
            "idx": np.ascontiguousarray(w[BL * i:BL * (i + 1)]),
        })
    return in_maps


_PROGRAM = None


def _build_program():
    global _PROGRAM
    if _PROGRAM is not None:
        return _PROGRAM
    nc = bacc.Bacc("TRN2", target_bir_lowering=False, debug=False,
                   enable_asserts=False, num_devices=NCORES,
                   num_swdge_queues=4)
    ys_t = nc.dram_tensor("ys", [BL, K, D, L], F32, kind="ExternalInput")
    idx_t = nc.dram_tensor("idx", [BL, K2, 128, NW], I16, kind="ExternalInput")
    out_t = nc.dram_tensor("out", [BL, D, L], F32, kind="ExternalOutput")
    with tile.TileContext(nc) as tc:
        crossmerge_body(tc, out_t.ap(), ys_t.ap(), idx_t.ap())
    nc.compile()
    _PROGRAM = nc
    return nc


def kernel(ys, vec_indices):
    nc = _build_program()
    in_maps = _host_prep(ys, vec_indices)
    res = run_bass_kernel_spmd(nc, in_maps, list(range(NCORES)))
    out = np.concatenate([r["out"] for r in res.results], axis=0)
    return out


# revision 25
# speedup vs baseline: 1.2295x; 1.2295x over previous
"""CrossMerge kernel for Trainium2 (8 NeuronCores, data-parallel over batch).

Computation (per batch b):
    inv[k]  = stable argsort of vec_indices[b, :, k]              (k = 0, 1)
    s_k[u,d] = ys[b,k,d,u] + ys[b,k+2,d,L-1-u]   (fwd + flipped half, pre-summed)
    out[b,d,l] = sum_k s_k[inv[k][l], d]

All-SBUF design (no DRAM scratch roundtrip, bit-exact f32):
  Phase A (per b,k): stream ys[b,k]/[b,k+2] in l-column chunks, one vector
           add per chunk (second operand read reversed), TensorE-transpose
           128-wide l tiles into PSUM [l,d] f32, then split every f32 into
           its two 16-bit halves while copying PSUM->SBUF into a token-major
           table: token u at partition u%128, rank u//128, payload
           [lo d0..127 | hi d0..127 | lo d128..191 pad | hi d128..191 pad]
           (512 int16 units = 1024B). Pure byte movement -> exact.
  Phase B (per b,k): SBUF-source dma_gather with transpose=True pulls the
           permuted tokens straight from the SBUF table; the DMA transpose
           unit (16-bit granularity) lands unit c*128+p on partition p,
           chunk c - so the lo/hi planes come out partition-aligned in
           [d, l] orientation. No second TensorE transpose pass needed.
  Phase C: re-interleave lo/hi planes into f32 bit patterns (strided i16
           copies), one f32 add sums the k pair, store [d, l] slices.

Host does only sharding + argsort-derived index prep; all tensor data
movement/compute is on device.
"""
import sys

sys.path.insert(0, "/opt/trn_rl_repo")

import numpy as np

import concourse.bacc as bacc
import concourse.bass as bass
import concourse.mybir as mybir
import concourse.tile as tile
from concourse.bass_utils import run_bass_kernel_spmd
from concourse.masks import make_identity

# Problem constants (hardcoded per contract).
B, K, D, H, W = 16, 4, 192, 56, 56
L = H * W          # 3136
K2 = K // 2        # 2
NCORES = 8
BL = B // NCORES   # 2 batches per core
C = 25             # l tiles of 128: 24 full + 1 of 64
LP = C * 128       # 3200 padded
U = 512            # int16 units per token payload (1024 bytes)
NW = LP // 16      # idx columns in wrapped int16 layout (200)
CHUNK = 512        # phase-A l-column streaming chunk (4 c-tiles)
F32 = mybir.dt.float32
I16 = mybir.dt.int16

# gather chunks: c-block ranges [0,7) [7,13) [13,19) [19,25)
CH_CLO = (0, 7, 13, 19)
CH_N = (896, 768, 768, 768)          # idx positions per chunk (%128 == 0)
CH_VALID = (896, 768, 768, 704)      # non-pad idx count per chunk
CH_L0 = (0, 896, 1664, 2432)         # output l offset per chunk


def crossmerge_body(tc, out_ap, ys_ap, idx_ap):
    """Tile kernel body.

    out_ap: [BL, 192, 3136] f32 (ExternalOutput)
    ys_ap:  [BL, 4, 192, 3136] f32 (ExternalInput)
    idx_ap: [BL, 2, 128, 200] i16, plain inv tokens, 16-wrapped, x8 groups
    """
    nc = tc.nc

    with (
        tc.tile_pool(name="const", bufs=1) as cpool,
        tc.tile_pool(name="idx", bufs=4) as ipool,
        tc.tile_pool(name="y", bufs=2) as ypool,
        tc.tile_pool(name="z", bufs=2) as zpool,
        tc.tile_pool(name="s", bufs=2) as spool,
        tc.tile_pool(name="tab", bufs=4) as tabpool,
        tc.tile_pool(name="g", bufs=8) as gpool,
        tc.tile_pool(name="x", bufs=4) as xpool,
        tc.tile_pool(name="xt", bufs=3) as xtpool,
        tc.tile_pool(name="ps", bufs=4, space="PSUM") as pspool,
    ):
        ident = cpool.tile([128, 128], F32)
        make_identity(nc, ident[:])

        # idx tiles first in the sync queue; gathers need them early
        idx_tiles = []
        for b in range(BL):
            i0 = ipool.tile([128, NW], I16, tag="idx")
            i1 = ipool.tile([128, NW], I16, tag="idx")
            nc.sync.dma_start(out=i0[:], in_=idx_ap[b, 0])
            nc.sync.dma_start(out=i1[:], in_=idx_ap[b, 1])
            idx_tiles.append((i0, i1))

        tabs = {}

        def phase_a(b, k):
            """Build the SBUF token table for (b, k)."""
            tab = tabpool.tile([128, C * U], I16, tag="tab")
            tabs[(b, k)] = tab
            tabv = tab[:].rearrange("p (c u) -> p c u", u=U)
            # the gather reads whole 1024B payloads; zero the pad units and
            # the never-written rank-24 rows so no uninit bytes are read
            nc.gpsimd.memset(tabv[:, :, 320:384], 0.0)
            nc.gpsimd.memset(tabv[:, :, 448:512], 0.0)
            nc.gpsimd.memset(tabv[64:, C - 1, :], 0.0)
            ys_f = ys_ap[b, k].rearrange("(g p) l -> p g l", p=96)
            ys_r = ys_ap[b, k + K2].rearrange("(g p) l -> p g l", p=96)
            for lo in range(0, L, CHUNK):
                hi = min(lo + CHUNK, L)
                w = hi - lo
                yc = ypool.tile([96, 2 * w], F32, tag="y")
                zc = zpool.tile([96, 2 * w], F32, tag="z")
                ycv = yc[:].rearrange("p (g l) -> p g l", g=2)
                zcv = zc[:].rearrange("p (g l) -> p g l", g=2)
                nc.sync.dma_start(out=ycv, in_=ys_f[:, :, lo:hi])
                nc.sync.dma_start(out=zcv, in_=ys_r[:, :, L - hi:L - lo])
                # s[:, g, l] = y[:, g, l] + z[:, g, L-1-l]
                sc = spool.tile([96, 2 * w], F32, tag="s")
                scv = sc[:].rearrange("p (g l) -> p g l", g=2)
                nc.vector.tensor_add(out=scv, in0=ycv, in1=zcv[:, :, ::-1])
                c0 = lo // 128
                ncs = (w + 127) // 128
                for j0 in range(0, ncs, 2):
                    gn = min(2, ncs - j0)
                    cc = c0 + j0
                    # all c-tiles in a 2-group are full except c=24 (alone)
                    szg = min(128, L - cc * 128)
                    ps = pspool.tile([128, 2, 192], F32, space="PSUM")
                    for j in range(gn):
                        c = cc + j
                        off = c * 128 - lo
                        for g in range(2):
                            nc.tensor.transpose(
                                out=ps[:szg, j, 96 * g:96 * (g + 1)],
                                in_=scv[:, g, off:off + szg],
                                identity=ident[:96, :96],
                            )
                    # split f32 -> (lo16, hi16) while copying PSUM -> table;
                    # on the scalar engine: int16 roundtrips the activation
                    # f32 path exactly, and it keeps the vector engine free
                    psi = ps[:].bitcast(I16)  # [128, 2, 384]: x = 2d+parity
                    nc.scalar.copy(
                        out=tabv[:szg, cc:cc + gn, 0:128],
                        in_=psi[:szg, :gn, 0:256:2])
                    nc.scalar.copy(
                        out=tabv[:szg, cc:cc + gn, 128:256],
                        in_=psi[:szg, :gn, 1:256:2])
                    nc.scalar.copy(
                        out=tabv[:szg, cc:cc + gn, 256:320],
                        in_=psi[:szg, :gn, 256:384:2])
                    nc.scalar.copy(
                        out=tabv[:szg, cc:cc + gn, 384:448],
                        in_=psi[:szg, :gn, 257:384:2])

        gs = {}

        def phase_b(b, k):
            """Issue the 4 chunked SBUF-source gathers for (b, k).

            All transpose-gathers go on ONE SWDGE queue: concurrent
            multi-queue transpose drains corrupt the shared transpose RX
            unit on this runtime (verified empirically).
            """
            it = idx_tiles[b][k]
            for ch in range(4):
                n = CH_N[ch]
                col0 = CH_L0[ch] // 16
                gt = gpool.tile([128, 4 * n], I16, tag="g")
                gv = gt[:].rearrange("p (c i) -> p c i", c=4)
                nc.gpsimd.dma_gather(
                    out_ap=gv,
                    in_ap=tabs[(b, k)][:],
                    idxs_ap=it[:, col0:col0 + n // 16],
                    num_idxs=n, num_idxs_reg=CH_VALID[ch], elem_size=U,
                    transpose=True, single_packet=True, queue_num=0,
                    sbuf_tokens_per_rank=128,
                    sbuf_free_dim_per_rank=2 * U)
                gs[(b, k, ch)] = gt

        def phase_c(b):
            """Recombine lo/hi planes, sum the k pair, store."""
            ov = out_ap[b]
            for ch in range(4):
                n, valid, l0 = CH_N[ch], CH_VALID[ch], CH_L0[ch]
                parts = []
                for k in range(K2):
                    gv = gs[(b, k, ch)][:].rearrange("p (c i) -> p c i", c=4)
                    xm = xpool.tile([128, n], F32, tag="x")
                    xt = xtpool.tile([64, n], F32, tag="xt")
                    xmv = xm[:].bitcast(I16).rearrange(
                        "p (i t) -> p i t", t=2)
                    xtv = xt[:].bitcast(I16).rearrange(
                        "p (i t) -> p i t", t=2)
                    nc.vector.tensor_copy(out=xmv[:, :, 0], in_=gv[:, 0, :])
                    nc.vector.tensor_copy(out=xmv[:, :, 1], in_=gv[:, 1, :])
                    nc.vector.tensor_copy(out=xtv[:, :, 0], in_=gv[:64, 2, :])
                    nc.vector.tensor_copy(out=xtv[:, :, 1], in_=gv[:64, 3, :])
                    parts.append((xm, xt))
                (x0, t0), (x1, t1) = parts
                nc.vector.tensor_add(out=x0[:], in0=x0[:], in1=x1[:])
                nc.vector.tensor_add(out=t0[:], in0=t0[:], in1=t1[:])
                nc.scalar.dma_start(
                    out=ov[0:128, l0:l0 + valid], in_=x0[:, :valid])
                nc.scalar.dma_start(
                    out=ov[128:192, l0:l0 + valid], in_=t0[:, :valid])

        # schedule: descgen for (b,k) directly after its table so the
        # gpsimd descgen stream overlaps the next table's build
        phase_a(0, 0)
        phase_b(0, 0)
        phase_a(0, 1)
        phase_b(0, 1)
        phase_a(1, 0)
        phase_b(1, 0)
        phase_c(0)
        phase_a(1, 1)
        phase_b(1, 1)
        phase_c(1)


def _host_prep(ys, vec_indices):
    """Shard inputs and build gather index tensors."""
    ys = np.ascontiguousarray(np.asarray(ys, dtype=np.float32)).reshape(
        B, K, D, L)
    vi = np.asarray(vec_indices)
    inv = np.argsort(vi, axis=1, kind="stable")          # [B, L, K2]
    invt = np.transpose(inv, (0, 2, 1)).astype(np.int16)  # [B, K2, L]
    # pad to 3200 with -1 (trailing pad: gathered garbage is never stored),
    # wrap in 16 partitions, replicate to the 8 gpsimd core groups
    rpad = np.concatenate(
        [invt, np.full((B, K2, LP - L), -1, dtype=np.int16)], axis=2)
    w = rpad.reshape(B, K2, NW, 16).transpose(0, 1, 3, 2)  # [B, K2, 16, NW]
    w = np.tile(w, (1, 1, 8, 1))                           # [B, K2, 128, NW]
    in_maps = []
    for i in range(NCORES):
        in_maps.append({
            "ys": ys[BL * i:BL * (i + 1)],
            "idx": np.ascontiguousarray(w[BL * i:BL * (i + 1)]),
        })
    return in_maps


_PROGRAM = None


def _build_program():
    global _PROGRAM
    if _PROGRAM is not None:
        return _PROGRAM
    nc = bacc.Bacc("TRN2", target_bir_lowering=False, debug=False,
                   enable_asserts=False, num_devices=NCORES,
                   num_swdge_queues=4)
    ys_t = nc.dram_tensor("ys", [BL, K, D, L], F32, kind="ExternalInput")
    idx_t = nc.dram_tensor("idx", [BL, K2, 128, NW], I16, kind="ExternalInput")
    out_t = nc.dram_tensor("out", [BL, D, L], F32, kind="ExternalOutput")
    with tile.TileContext(nc) as tc:
        crossmerge_body(tc, out_t.ap(), ys_t.ap(), idx_t.ap())
    nc.compile()
    _PROGRAM = nc
    return nc


def kernel(ys, vec_indices):
    nc = _build_program()
    in_maps = _host_prep(ys, vec_indices)
    res = run_bass_kernel_spmd(nc, in_maps, list(range(NCORES)))
    out = np.concatenate([r["out"] for r in res.results], axis=0)
    return out
